# revision 1
# baseline (speedup 1.0000x reference)
"""TRN2 Bass kernel: transformer Block (LN->MHA->2x residual->LN->MLP) for
B=32,N=512,C=768,H=12. Data-parallel over batch across 8 NeuronCores (4
items/core). All matmuls run on the PE in float32r (full-rate fp32 mode,
1 cyc/row at N>=256).

Per-core program:
  prologue: PE-transpose qkv/proj weights into [c-on-partition] layout
  phase 1 (per batch item): LN1 -> h0 -> PE-transpose -> qkT/v matmuls ->
    per-head scoresT = kT.T@qT -> exp (no max-sub; scores are N(0,1)-scale) ->
    [v|1]-augmented AV matmul (oT + softmax denominators in one pass) ->
    normalize via reciprocal + PE-broadcast -> proj -> x2=2*(proj+proj_b) ->
    spill x2 to DRAM
  phase 2a (t-chunks of 512): LN2 -> h2T -> fc1 -> gelu -> spill fc1outT
  phase 2b (t-chunks of 512): fc2 -> + x2 + fc2_b -> out
"""
import json
import os
import tempfile

import numpy as np
from contextlib import ExitStack

import concourse.bass as bass
import concourse.tile as tile
import concourse.bacc as bacc
from concourse import mybir
from concourse.bass_utils import run_bass_kernel_spmd
from concourse.masks import make_identity

F32 = mybir.dt.float32
F32R = mybir.dt.float32r
AF = mybir.ActivationFunctionType
ALU = mybir.AluOpType

B, N, C = 32, 512, 768
H, D = 12, 64
HID = 4 * C
EPS = 1e-5
NCORES = 8
BPC = B // NCORES            # batch items per core
T = BPC * N                  # tokens per core
CK = C // 128                # 6 contraction chunks over C
FQK = (2 * C) // 128         # 12 feature tiles for q+k
JH = HID // 128              # 24 hidden feature tiles
NT = N // 128                # 4 token tiles per item
SCALE = D ** -0.5
TC2 = 512                    # phase-2 token chunk


def _bc(ap, p=128):
    """Broadcast a 1-D DRAM AP across p partitions (stride-0 partition dim)."""
    return bass.AP(tensor=ap.tensor, offset=ap.offset, ap=[[0, p]] + list(ap.ap))


def _emit(tc, io, ctx):
    nc = tc.nc

    consts = ctx.enter_context(tc.tile_pool(name="consts", bufs=1))
    wbig = ctx.enter_context(tc.tile_pool(name="wbig", bufs=1))
    small = ctx.enter_context(tc.tile_pool(name="small", bufs=4))
    xio = ctx.enter_context(tc.tile_pool(name="xio", bufs=2))
    ps1 = ctx.enter_context(tc.tile_pool(name="ps1", bufs=4, space="PSUM"))
    ps2 = ctx.enter_context(tc.tile_pool(name="ps2", bufs=2, space="PSUM"))
    dram = ctx.enter_context(tc.tile_pool(name="dram", bufs=1, space="DRAM"))

    # ---------------- constants ----------------
    ident32 = consts.tile([128, 128], F32)
    make_identity(nc, ident32)
    identr = consts.tile([128, 128], F32R)
    nc.vector.tensor_copy(out=identr, in_=ident32)
    onesf2 = consts.tile([128, 64], F32)
    nc.vector.memset(onesf2, 1.0)
    onesr = consts.tile([128, 64], F32R)
    nc.vector.tensor_copy(out=onesr, in_=onesf2)
    onecol = consts.tile([128, NT * H], F32)
    nc.vector.memset(onecol, 1.0)
    epst = consts.tile([128, 1], F32)
    nc.vector.memset(epst, EPS)

    ln1w_bc = consts.tile([128, C], F32)
    nc.sync.dma_start(out=ln1w_bc, in_=_bc(io["ln1_w"]))
    ln1b_bc = consts.tile([128, C], F32)
    nc.sync.dma_start(out=ln1b_bc, in_=_bc(io["ln1_b"]))
    ln2w_bc = consts.tile([128, C], F32)
    nc.sync.dma_start(out=ln2w_bc, in_=_bc(io["ln2_w"]))
    ln2b_bc = consts.tile([128, C], F32)
    nc.sync.dma_start(out=ln2b_bc, in_=_bc(io["ln2_b"]))
    pb2_bc = consts.tile([128, C], F32)
    nc.sync.dma_start(out=pb2_bc, in_=_bc(io["proj_b"]))
    nc.scalar.mul(out=pb2_bc, in_=pb2_bc, mul=2.0)
    fc2b_bc = consts.tile([128, C], F32)
    nc.sync.dma_start(out=fc2b_bc, in_=_bc(io["fc2_b"]))
    fc1b_t = consts.tile([128, JH], F32)
    nc.sync.dma_start(out=fc1b_t, in_=io["fc1_b"].rearrange("(j p) -> p j", p=128))

    # DRAM scratch
    x2d = dram.tile([T, C], F32)
    f1d = dram.tile([JH, 128, T], F32R)

    # ---------------- weight transposition helper ----------------
    evac_ctr = [0]

    def load_wT(w_ap, nrows, ncols, dst, stg):
        """w [nrows, ncols] row-major DRAM -> dst [128, ncols//128, nrows] F32R."""
        nj, nk = nrows // 128, ncols // 128
        wr = w_ap.rearrange("(j p) c -> p j c", p=128)
        for j in range(nj):
            for c0 in range(0, nk, 6):
                cn = min(6, nk - c0)
                piece = stg.tile([128, 768], F32, tag="wstage", name="piece")
                nc.sync.dma_start(out=piece[:, 0:cn * 128],
                                  in_=wr[:, j, c0 * 128:(c0 + cn) * 128])
                for k in range(cn):
                    tp = ps1.tile([128, 128], F32, tag="s1", name="tp")
                    nc.tensor.transpose(tp[:], piece[:, k * 128:(k + 1) * 128],
                                        ident32[:])
                    if evac_ctr[0] % 2 == 0:
                        nc.vector.tensor_copy(
                            out=dst[:, c0 + k, j * 128:(j + 1) * 128], in_=tp[:])
                    else:
                        nc.scalar.copy(
                            out=dst[:, c0 + k, j * 128:(j + 1) * 128], in_=tp[:])
                    evac_ctr[0] += 1

    def layer_norm(x_t, w_bcast, b_bcast, pool):
        """x_t [128, C] f32 -> returns h [128, C] F32R = LN(x)*w + b."""
        st = small.tile([128, 3, nc.vector.BN_STATS_DIM], F32, tag="bnst",
                        name="st")
        for i in range(3):
            nc.vector.bn_stats(out=st[:, i, :], in_=x_t[:, 256 * i:256 * (i + 1)])
        mv = small.tile([128, nc.vector.BN_AGGR_DIM], F32, tag="mv", name="mv")
        nc.vector.bn_aggr(out=mv, in_=st)
        rstd = small.tile([128, 1], F32, tag="rstd", name="rstd")
        nc.scalar.activation(out=rstd, in_=mv[:, 1:2], func=AF.Sqrt, bias=epst)
        nc.vector.reciprocal(out=rstd, in_=rstd)
        ht = pool.tile([128, C], F32, tag="lnt", bufs=1, name="ht")
        nc.vector.tensor_scalar(out=ht, in0=x_t, scalar1=mv[:, 0:1],
                                scalar2=rstd, op0=ALU.subtract, op1=ALU.mult)
        nc.vector.tensor_mul(out=ht, in0=ht, in1=w_bcast)
        h = pool.tile([128, C], F32R, tag="h0", bufs=1, name="h")
        nc.vector.tensor_add(out=h, in0=ht, in1=b_bcast)
        return h

    def transpose_to(h, dstT, tt):
        """h [128, C] F32R -> dstT[:, k, tt*128:(tt+1)*128] for k in CK."""
        for k in range(CK):
            tp = ps2.tile([128, 128], F32R, tag="s2", name="tp")
            nc.tensor.transpose(tp[:], h[:, k * 128:(k + 1) * 128], identr[:])
            if k % 2 == 0:
                nc.vector.tensor_copy(
                    out=dstT[:, k, tt * 128:(tt + 1) * 128], in_=tp[:])
            else:
                nc.scalar.copy(
                    out=dstT[:, k, tt * 128:(tt + 1) * 128], in_=tp[:])

    # ================= stage A: weights + phase 1 =================
    with tc.tile_pool(name="wstage_a", bufs=2) as wstage_a, \
         tc.tile_pool(name="wp", bufs=1) as wp_pool, \
         tc.tile_pool(name="p1", bufs=1) as p1:

        wqkvT = wbig.tile([128, CK, 3 * C], F32R, tag="w")
        load_wT(io["qkv_w"], 3 * C, C, wqkvT, wstage_a)
        wpT = wp_pool.tile([128, CK, C], F32R)
        load_wT(io["proj_w"], C, C, wpT, wstage_a)

        for b in range(BPC):
            t0 = b * N
            h0T = p1.tile([128, CK, N], F32R, tag="h0T", name="h0T")
            for tt in range(NT):
                x_t = xio.tile([128, C], F32, tag="xio", name="x_t")
                nc.sync.dma_start(
                    out=x_t, in_=io["x"][t0 + tt * 128:t0 + (tt + 1) * 128, :])
                h0 = layer_norm(x_t, ln1w_bc, ln1b_bc, p1)
                transpose_to(h0, h0T, tt)

            # qkT: feature tile j holds heads 2j / 2j+1 stacked on partitions
            qk_sb = p1.tile([128, FQK, N], F32R, tag="qk", name="qk_sb")
            for j in range(FQK):
                qp = ps1.tile([128, N], F32, tag="s1", name="qp")
                for k in range(CK):
                    nc.tensor.matmul(qp[:], wqkvT[:, k, j * 128:(j + 1) * 128],
                                     h0T[:, k, :], start=(k == 0),
                                     stop=(k == CK - 1))
                nc.scalar.copy(out=qk_sb[:, j, :], in_=qp[:])

            # v (tokens on partitions) with ones column at d=D
            v_sb = p1.tile([128, NT, H, D + 1], F32R, tag="v", name="v_sb")
            nc.vector.tensor_copy(
                out=v_sb[:, :, :, D:D + 1],
                in_=onecol.rearrange("p (a b c) -> p a b c", a=NT, b=H))
            for tt in range(NT):
                vp = ps2.tile([128, C], F32, tag="s2", name="vp")
                for k in range(CK):
                    for half, n0, nn in ((0, 0, 512), (1, 512, 256)):
                        nc.tensor.matmul(vp[:, n0:n0 + nn],
                                         h0T[:, k, tt * 128:(tt + 1) * 128],
                                         wqkvT[:, k, 2 * C + n0:2 * C + n0 + nn],
                                         start=(k == 0), stop=(k == CK - 1))
                nc.vector.tensor_copy(out=v_sb[:, tt, :, 0:D],
                                      in_=vp.rearrange("p (h d) -> p h d", h=H))

            # attention; oT: head h -> chunk h//2, partitions 64*(h%2)
            oT = p1.tile([128, CK, N], F32R, tag="oT", name="oT")
            for q4 in range(H // 4):
                srow = p1.tile([128, N], F32, tag="srow", bufs=2, name="srow")
                nc.vector.memset(srow, 1.0)
                orws = []
                for pi in range(2):
                    hp = 2 * q4 + pi
                    kj = FQK // 2 + hp
                    orw = p1.tile([128, N], F32, tag="orw", bufs=2, name="orw")
                    for sub in range(2):
                        h = 2 * hp + sub
                        p0 = 64 * sub
                        r = 32 * (h % 4)
                        av = ps1.tile([D + 1, N], F32, tag="s1", name="av")
                        for c in range(NT):
                            sc = ps1.tile([128, N], F32, tag="s1", name="sc")
                            nc.tensor.matmul(
                                sc[:],
                                qk_sb[p0:p0 + D, kj, c * 128:(c + 1) * 128],
                                qk_sb[p0:p0 + D, hp, :])
                            ex = p1.tile([128, N], F32R, tag="e5", bufs=2,
                                         name="ex")
                            nc.scalar.activation(out=ex, in_=sc[:], func=AF.Exp,
                                                 scale=SCALE)
                            nc.tensor.matmul(av[:], v_sb[:, c, h, :], ex[:],
                                             start=(c == 0), stop=(c == NT - 1))
                        # gather sums at 32-aligned rows; stash o rows
                        # (on DVE: ScalarE's exp gates the AV critical path)
                        nc.vector.tensor_copy(out=srow[r:r + 1, :],
                                              in_=av[D:D + 1, :])
                        nc.vector.tensor_copy(out=orw[p0:p0 + D, :],
                                              in_=av[0:D, :])
                    orws.append(orw)
                # one batched reciprocal for 4 heads (DVE div is 8 cyc/elem)
                rec4 = p1.tile([128, N], F32R, tag="srow", bufs=2, name="rec4")
                with nc.allow_low_precision(reason="softmax denom recip"):
                    nc.vector.reciprocal(out=rec4[0:97, 0:N // 2],
                                         in_=srow[0:97, 0:N // 2])
                    nc.vector.reciprocal(out=rec4[0:97, N // 2:N],
                                         in_=srow[0:97, N // 2:N])
                for pi in range(2):
                    hp = 2 * q4 + pi
                    for sub in range(2):
                        p0 = 64 * sub
                        r = 32 * ((2 * pi + sub) % 4)
                        bcp = ps1.tile([64, N], F32, tag="s1", name="bcp")
                        for n0 in (0, N // 2):
                            nc.tensor.matmul(bcp[:, n0:n0 + N // 2],
                                             onesr[r:r + 1, 0:64],
                                             rec4[r:r + 1, n0:n0 + N // 2],
                                             tile_position=(r, 0))
                        nc.vector.tensor_mul(out=oT[p0:p0 + D, hp, :],
                                             in0=bcp[:],
                                             in1=orws[pi][p0:p0 + D, :])

            # proj + double + spill x2
            for tt in range(NT):
                pr = ps2.tile([128, C], F32, tag="s2", name="pr")
                for k in range(CK):
                    for half, n0, nn in ((0, 0, 512), (1, 512, 256)):
                        nc.tensor.matmul(pr[:, n0:n0 + nn],
                                         oT[:, k, tt * 128:(tt + 1) * 128],
                                         wpT[:, k, n0:n0 + nn],
                                         start=(k == 0), stop=(k == CK - 1))
                x2a = xio.tile([128, C], F32, tag="x2s", name="x2a")
                nc.scalar.mul(out=x2a, in_=pr[:], mul=2.0)
                x2t = xio.tile([128, C], F32, tag="x2s", name="x2t")
                nc.gpsimd.tensor_add(out=x2t, in0=x2a, in1=pb2_bc)
                nc.scalar.dma_start(
                    out=x2d[t0 + tt * 128:t0 + (tt + 1) * 128, :], in_=x2t)

    # ================= stage B: fc1 =================
    with tc.tile_pool(name="wstage_b", bufs=2) as wstage_b, \
         tc.tile_pool(name="p2a", bufs=1) as p2a:
        wf1T = wbig.tile([128, CK, HID], F32R, tag="w")
        load_wT(io["fc1_w"], HID, C, wf1T, wstage_b)

        h2T = p2a.tile([128, CK, T], F32R, tag="h2T", name="h2T")
        for tt in range(T // 128):
            x2_t = xio.tile([128, C], F32, tag="xio", name="x2_t")
            nc.sync.dma_start(
                out=x2_t, in_=x2d[tt * 128:(tt + 1) * 128, :])
            h2 = layer_norm(x2_t, ln2w_bc, ln2b_bc, p2a)
            transpose_to(h2, h2T, tt)
        NQ = T // 512
        for j in range(JH):
            fps = [ps1.tile([128, 512], F32, tag="s1", name="fp")
                   for _ in range(NQ)]
            for k in range(CK):
                for q in range(NQ):
                    nc.tensor.matmul(fps[q][:],
                                     wf1T[:, k, j * 128:(j + 1) * 128],
                                     h2T[:, k, q * 512:(q + 1) * 512],
                                     start=(k == 0), stop=(k == CK - 1))
            for q in range(NQ):
                g = p2a.tile([128, 512], F32R, tag="gel", bufs=4, name="g")
                nc.scalar.activation(out=g, in_=fps[q][:], func=AF.Gelu,
                                     bias=fc1b_t[:, j:j + 1])
                nc.scalar.dma_start(out=f1d[j, :, q * 512:(q + 1) * 512],
                                    in_=g)

    # ================= stage C: fc2 + residual =================
    with tc.tile_pool(name="wstage_c", bufs=2) as wstage_c, \
         tc.tile_pool(name="p2b", bufs=1) as p2b:
        wf2T = wbig.tile([128, JH, C], F32R, tag="w")
        load_wT(io["fc2_w"], C, HID, wf2T, wstage_c)

        for ch in range(T // TC2):
            t0 = ch * TC2
            f1h = []
            for hf in range(2):
                f1t = p2b.tile([128, JH // 2, TC2], F32R, tag="f1in", bufs=3,
                               name="f1t")
                nc.sync.dma_start(
                    out=f1t,
                    in_=f1d[hf * (JH // 2):(hf + 1) * (JH // 2),
                            :, t0:t0 + TC2].rearrange("j p t -> p j t"))
                f1h.append(f1t)
            for tt in range(TC2 // 128):
                x2_t = xio.tile([128, C], F32, tag="xio", name="x2_t")
                nc.sync.dma_start(
                    out=x2_t, in_=x2d[t0 + tt * 128:t0 + (tt + 1) * 128, :])
                x2pb = xio.tile([128, C], F32, tag="xio", name="x2pb")
                nc.vector.tensor_add(out=x2pb, in0=x2_t, in1=fc2b_bc)
                f2 = ps2.tile([128, C], F32, tag="s2", name="f2")
                for k in range(JH):
                    for half, n0, nn in ((0, 0, 512), (1, 512, 256)):
                        nc.tensor.matmul(f2[:, n0:n0 + nn],
                                         f1h[k // (JH // 2)][
                                             :, k % (JH // 2),
                                             tt * 128:(tt + 1) * 128],
                                         wf2T[:, k, n0:n0 + nn],
                                         start=(k == 0), stop=(k == JH - 1))
                o_t = p2b.tile([128, C], F32, tag="outt", bufs=2, name="o_t")
                nc.vector.tensor_add(out=o_t, in0=f2[:], in1=x2pb)
                nc.sync.dma_start(
                    out=io["out"][t0 + tt * 128:t0 + (tt + 1) * 128, :], in_=o_t)


_CACHE = {}


def _act_table_override():
    """Drop the exp-only / ln-only ACT table sets so walrus selects
    natural_log_exp_and_others — the kernel alternates Exp and Ln per head
    and per-LN-tile, and each table switch costs ~1.5us on ScalarE."""
    return  # any act-root override breaks NEFF exec on the axon terminal
    try:
        from neuronxcc.driver.Job import Job
        from neuronxcc.driver.jobs.support.FindActInfo import findActInfoFile
        orig = findActInfoFile(Job.getPackageDir(), "gen3")
        d = json.load(open(orig))
        pref = [s for s in d["act_func_sets"]
                if s["name"] == "natural_log_exp_and_others"]
        rest = [s for s in d["act_func_sets"]
                if s["name"] != "natural_log_exp_and_others"]
        d["act_func_sets"] = pref + rest
        tmp = tempfile.mkdtemp(prefix="act_override_")
        src_dir = os.path.dirname(orig)
        base = os.path.basename(orig)
        for f in os.listdir(src_dir):
            if f != base:
                os.symlink(os.path.join(src_dir, f), os.path.join(tmp, f))
        path = os.path.join(tmp, base)
        with open(path, "w") as fh:
            json.dump(d, fh)
        os.environ["BASS_ACT_ROOT_JSON_PATH"] = path
    except Exception:
        pass


def _build():
    if "nc" in _CACHE:
        return _CACHE["nc"]
    _act_table_override()
    nc = bacc.Bacc("TRN2", target_bir_lowering=False, debug=False,
                   num_devices=NCORES)
    io = {}
    io["x"] = nc.dram_tensor("x", [T, C], F32, kind="ExternalInput").ap()
    for name, shape in [("ln1_w", [C]), ("ln1_b", [C]), ("qkv_w", [3 * C, C]),
                        ("proj_w", [C, C]), ("proj_b", [C]), ("ln2_w", [C]),
                        ("ln2_b", [C]), ("fc1_w", [HID, C]), ("fc1_b", [HID]),
                        ("fc2_w", [C, HID]), ("fc2_b", [C])]:
        io[name] = nc.dram_tensor(name, shape, F32, kind="ExternalInput").ap()
    io["out"] = nc.dram_tensor("out", [T, C], F32, kind="ExternalOutput").ap()

    with tile.TileContext(nc) as tc:
        with ExitStack() as ctx:
            _emit(tc, io, ctx)
    nc.compile()
    _CACHE["nc"] = nc
    return nc


def kernel(**inputs):
    nc = _build()
    arrs = {k: np.ascontiguousarray(np.asarray(v, dtype=np.float32))
            for k, v in inputs.items()}
    x = arrs.pop("x").reshape(B, N, C)
    in_maps = []
    for c in range(NCORES):
        m = dict(arrs)
        m["x"] = np.ascontiguousarray(x[c * BPC:(c + 1) * BPC].reshape(T, C))
        in_maps.append(m)
    res = run_bass_kernel_spmd(nc, in_maps, core_ids=list(range(NCORES)))
    out = np.concatenate(
        [r["out"].reshape(BPC, N, C) for r in res.results], axis=0)
    return out.astype(np.float32)


if __name__ == "__main__":
    rng = np.random.default_rng(0)
    ins = {
        "x": rng.standard_normal((B, N, C), dtype=np.float32),
        "ln1_w": np.ones(C, np.float32), "ln1_b": np.zeros(C, np.float32),
        "qkv_w": rng.standard_normal((3 * C, C), dtype=np.float32) / np.sqrt(C),
        "proj_w": rng.standard_normal((C, C), dtype=np.float32) / np.sqrt(C),
        "proj_b": np.zeros(C, np.float32),
        "ln2_w": np.ones(C, np.float32), "ln2_b": np.zeros(C, np.float32),
        "fc1_w": rng.standard_normal((HID, C), dtype=np.float32) / np.sqrt(C),
        "fc1_b": np.zeros(HID, np.float32),
        "fc2_w": rng.standard_normal((C, HID), dtype=np.float32) / np.sqrt(HID),
        "fc2_b": np.zeros(C, np.float32),
    }
    out = kernel(**ins)
    print("out", out.shape, out.dtype, np.abs(out).max())



# revision 17
# speedup vs baseline: 1.2063x; 1.2063x over previous
"""TRN2 Bass kernel: transformer Block (LN->MHA->2x residual->LN->MLP) for
B=32,N=512,C=768,H=12. Data-parallel over batch across 8 NeuronCores (4
items/core). Fully fused per-item pipeline, all weights resident in SBUF as
bf16, no DRAM spills.

Per-core program:
  prologue: PE-transpose all weights into [c-on-partition] bf16 layout; fold
    ln1_w/ln2_w into qkv_w/fc1_w; fold ln biases into matmul biases.
  per item b (software-pipelined: attn(b+1) emitted before mlp_rest(b)):
    LN1 -> h0 (bf16) -> PE-transpose -> qk/v matmuls -> per-head
    scoresT = kT.T@qT -> exp (ScalarE, 1024-wide from PSUM, no max-sub) ->
    [v|1]-augmented AV matmul -> denom reciprocal via Ln+Exp on ScalarE
    (same ACT table set as Exp) -> PE-broadcast -> oT -> proj ->
    x2=2*(proj+proj_b) in SBUF -> LN2 -> h2T -> per-256-token-half:
    fc1 -> gelu -> fc2 -> + x2 + fc2_b -> out
"""
import numpy as np
from contextlib import ExitStack

import concourse.bass as bass
import concourse.tile as tile
import concourse.bacc as bacc
from concourse import mybir
from concourse.bass_utils import run_bass_kernel_spmd
from concourse.masks import make_identity

F32 = mybir.dt.float32
BF16 = mybir.dt.bfloat16
AF = mybir.ActivationFunctionType
ALU = mybir.AluOpType

B, N, C = 32, 512, 768
H, D = 12, 64
HID = 4 * C
EPS = 1e-5
NCORES = 8
BPC = B // NCORES            # batch items per core
T = BPC * N                  # tokens per core
CK = C // 128                # 6 contraction chunks over C
FQK = (2 * C) // 128         # 12 feature tiles for q+k
JH = HID // 128              # 24 hidden feature tiles
NT = N // 128                # 4 token tiles per item
SCALE = D ** -0.5


def _bc(ap, p=128):
    """Broadcast a 1-D DRAM AP across p partitions (stride-0 partition dim)."""
    return bass.AP(tensor=ap.tensor, offset=ap.offset, ap=[[0, p]] + list(ap.ap))


def _emit(tc, io, ctx):
    nc = tc.nc

    consts = ctx.enter_context(tc.tile_pool(name="consts", bufs=1))
    wbig = ctx.enter_context(tc.tile_pool(name="wbig", bufs=1))
    small = ctx.enter_context(tc.tile_pool(name="small", bufs=4))
    xio = ctx.enter_context(tc.tile_pool(name="xio", bufs=2))
    hln = ctx.enter_context(tc.tile_pool(name="hln", bufs=2))
    hTp = ctx.enter_context(tc.tile_pool(name="hTp", bufs=3))
    qkp = ctx.enter_context(tc.tile_pool(name="qkp", bufs=1))
    vp_ = ctx.enter_context(tc.tile_pool(name="vp", bufs=1))
    exp_ = ctx.enter_context(tc.tile_pool(name="exp", bufs=2))
    zp = ctx.enter_context(tc.tile_pool(name="zp", bufs=2))
    orp = ctx.enter_context(tc.tile_pool(name="orp", bufs=2))
    x2p = ctx.enter_context(tc.tile_pool(name="x2p", bufs=2))
    gp = ctx.enter_context(tc.tile_pool(name="gp", bufs=1))
    outp = ctx.enter_context(tc.tile_pool(name="outp", bufs=2))
    psQP = ctx.enter_context(tc.tile_pool(name="psQP", bufs=2, space="PSUM"))
    psSC = ctx.enter_context(tc.tile_pool(name="psSC", bufs=2, space="PSUM"))
    psAV = ctx.enter_context(tc.tile_pool(name="psAV", bufs=2, space="PSUM"))

    # ---------------- constants ----------------
    ident32 = consts.tile([128, 128], F32)
    make_identity(nc, ident32)
    identb = consts.tile([128, 128], BF16)
    nc.vector.tensor_copy(out=identb, in_=ident32)
    onesb = consts.tile([97, 64], BF16)
    nc.vector.memset(onesb, 1.0)
    onescol = consts.tile([1, 128], BF16)
    nc.vector.memset(onescol, 1.0)
    onecol = consts.tile([128, NT * H], BF16)
    nc.vector.memset(onecol, 1.0)
    epst = consts.tile([128, 1], F32)
    nc.vector.memset(epst, EPS)

    # bias rows [1, C] (folded into PE accumulations via rank-1 ones-matmuls)
    brow = consts.tile([1, 2, C], BF16)
    for bi, bname in ((0, "proj_b"), (1, "fc2_b")):
        b_st = xio.tile([128, C], F32, tag="xio", name="b_st")
        nc.sync.dma_start(out=b_st[0:1, :], in_=_bc(io[bname], p=1))
        nc.vector.tensor_copy(out=brow[:, bi, :], in_=b_st[0:1, :])

    fc1b_t = consts.tile([128, JH], F32)
    ln1w_c = consts.tile([128, CK], F32)
    nc.sync.dma_start(out=ln1w_c, in_=io["ln1_w"].rearrange("(k p) -> p k", p=128))
    ln2w_c = consts.tile([128, CK], F32)
    nc.sync.dma_start(out=ln2w_c, in_=io["ln2_w"].rearrange("(k p) -> p k", p=128))
    lnb_st = consts.tile([128, 2, CK], F32)
    nc.sync.dma_start(out=lnb_st[:, 0, :],
                      in_=io["ln1_b"].rearrange("(k p) -> p k", p=128))
    nc.sync.dma_start(out=lnb_st[:, 1, :],
                      in_=io["ln2_b"].rearrange("(k p) -> p k", p=128))
    lnb_c = consts.tile([128, 2, CK], BF16)
    nc.vector.tensor_copy(out=lnb_c, in_=lnb_st)
    qkb = consts.tile([128, FQK], F32)
    vrow = consts.tile([1, C], BF16)

    # resident weights (bf16, contraction dim on partitions)
    wqkvT = wbig.tile([128, CK, 3 * C], BF16)
    wpT = wbig.tile([128, CK, C], BF16)
    wf1T = wbig.tile([128, CK, HID], BF16)
    wf2T = wbig.tile([128, JH, C], BF16)

    evac_ctr = [0]

    def load_wT(w_ap, nrows, ncols, dst):
        """w [nrows, ncols] row-major DRAM -> dst [128, ncols//128, nrows] BF16."""
        nj, nk = nrows // 128, ncols // 128
        wr = w_ap.rearrange("(j p) c -> p j c", p=128)
        for j in range(nj):
            for c0 in range(0, nk, 6):
                cn = min(6, nk - c0)
                piece = wstage.tile([128, 768], F32, tag="wstage", name="piece")
                nc.sync.dma_start(out=piece[:, 0:cn * 128],
                                  in_=wr[:, j, c0 * 128:(c0 + cn) * 128])
                for b0 in range(0, cn, 4):
                    bn_ = min(4, cn - b0)
                    tp = psQP.tile([128, bn_, 128], F32, tag="qp", name="tp")
                    for k in range(bn_):
                        nc.tensor.matmul(
                            tp[:, k, :], piece[:, (b0 + k) * 128:(b0 + k + 1) * 128],
                            ident32[:], is_transpose=True)
                    dstap = dst[:, c0 + b0:c0 + b0 + bn_, j * 128:(j + 1) * 128]
                    if evac_ctr[0] % 2 == 0:
                        nc.vector.tensor_copy(out=dstap, in_=tp[:])
                    else:
                        nc.scalar.copy(out=dstap, in_=tp[:])
                    evac_ctr[0] += 1

    def fold_lnw(dst, nf, lnw_col):
        """dst[:, k, :] *= lnw_col[:, k] for all k (per-partition scalar)."""
        for k in range(CK):
            nc.vector.tensor_scalar(out=dst[:, k, 0:nf], in0=dst[:, k, 0:nf],
                                    scalar1=lnw_col[:, k:k + 1], scalar2=None,
                                    op0=ALU.mult)

    # ---------------- per-item helpers ----------------
    def ln_center(x_t, pool, tag, bufs):
        """x_t [128, C] -> h [128, C] BF16 = (x - mu) * rsqrt(var + eps)."""
        st = small.tile([128, 3, nc.vector.BN_STATS_DIM], F32, tag="bnst",
                        name="st")
        for i in range(3):
            nc.vector.bn_stats(out=st[:, i, :], in_=x_t[:, 256 * i:256 * (i + 1)])
        mv = small.tile([128, nc.vector.BN_AGGR_DIM], F32, tag="mv", name="mv")
        nc.vector.bn_aggr(out=mv, in_=st)
        lnv = small.tile([128, 1], F32, tag="lnv", name="lnv")
        nc.scalar.activation(out=lnv, in_=mv[:, 1:2], func=AF.Ln, bias=epst)
        rstd = small.tile([128, 1], F32, tag="rstd", name="rstd")
        nc.scalar.activation(out=rstd, in_=lnv, func=AF.Exp, scale=-0.5)
        h = pool.tile([128, C], BF16, tag=tag, bufs=bufs, name="h")
        nc.vector.tensor_scalar(out=h, in0=x_t, scalar1=mv[:, 0:1],
                                scalar2=rstd, op0=ALU.subtract, op1=ALU.mult)
        return h

    def transpose_to(h, dstT, tt, who):
        """h [128, C] BF16 -> dstT[:, k, tt*128:(tt+1)*128] for k in CK."""
        for b0, bn_ in ((0, 4), (4, 2)):
            tp = psQP.tile([128, bn_, 128], BF16, tag="qp", name="tpb")
            for k in range(bn_):
                nc.tensor.matmul(tp[:, k, :],
                                 h[:, (b0 + k) * 128:(b0 + k + 1) * 128],
                                 identb[:], is_transpose=True)
            dstap = dstT[:, b0:b0 + bn_, tt * 128:(tt + 1) * 128]
            if who % 2 == 0:
                nc.vector.tensor_copy(out=dstap, in_=tp[:])
            else:
                nc.scalar.copy(out=dstap, in_=tp[:])

    def attn(b, h2T_of=None):
        """Attention for item b; also emits LN2+h2T for item h2T_of."""
        t0 = b * N
        h0T = hTp.tile([128, CK, N], BF16, tag="hT", name="h0T")
        for tt in range(NT):
            x_t = xio.tile([128, C], F32, tag="xio", name="x_t")
            nc.sync.dma_start(
                out=x_t, in_=io["x"][t0 + tt * 128:t0 + (tt + 1) * 128, :])
            h = ln_center(x_t, hln, "h0", 2)
            transpose_to(h, h0T, tt, tt)

        # qT/kT feature-major: tile j holds heads 2j/2j+1 stacked on partitions
        qk = qkp.tile([128, FQK, N], BF16, tag="qk", name="qk")
        for j in range(FQK):
            qp = psQP.tile([128, N], F32, tag="qp", name="qp")
            for k in range(CK):
                nc.tensor.matmul(qp[:], wqkvT[:, k, j * 128:(j + 1) * 128],
                                 h0T[:, k, :], start=(k == 0),
                                 stop=(k == CK - 1))
            if j % 2 == 0:
                nc.scalar.activation(out=qk[:, j, :], in_=qp[:],
                                     func=AF.Identity, bias=qkb[:, j:j + 1])
            else:
                nc.vector.tensor_scalar(out=qk[:, j, :], in0=qp[:],
                                        scalar1=qkb[:, j:j + 1], scalar2=None,
                                        op0=ALU.add)

        # v token-major with ones column at d=D
        v = vp_.tile([128, NT, H, D + 1], BF16, tag="v", name="v")
        nc.vector.tensor_copy(
            out=v[:, :, :, D:D + 1],
            in_=onecol.rearrange("p (a b c) -> p a b c", a=NT, b=H))
        for tt in range(NT):
            for n0, nn in ((0, 512), (512, 256)):
                vps = psQP.tile([128, nn], F32, tag="qp", name="vps")
                for k in range(CK):
                    nc.tensor.matmul(vps[:],
                                     h0T[:, k, tt * 128:(tt + 1) * 128],
                                     wqkvT[:, k, 2 * C + n0:2 * C + n0 + nn],
                                     start=(k == 0), stop=False)
                nc.tensor.matmul(vps[:], onescol[:, 0:128],
                                 vrow[:, n0:n0 + nn], start=False, stop=True)
                nc.vector.tensor_copy(
                    out=v[:, tt, n0 // D:(n0 + nn) // D, 0:D],
                    in_=vps.rearrange("p (h d) -> p h d", d=D))

        if h2T_of is not None:
            mlp_front(h2T_of)

        # attention: head h -> oT chunk h//2, partitions 64*(h%2)
        oT = hTp.tile([128, CK, N], BF16, tag="hT", name="oT")
        for q4 in range(H // 4):
            zr = zp.tile([97, N], F32, tag="zr", name="zr")
            orws = []
            for pi in range(2):
                hp = 2 * q4 + pi
                kj = FQK // 2 + hp
                orw = orp.tile([128, N], BF16, tag="orw", name="orw")
                for sub in range(2):
                    h = 2 * hp + sub
                    p0 = 64 * sub
                    r = 32 * (2 * pi + sub)
                    av = psAV.tile([D + 1, N], F32, tag="av", name="av")
                    for ci in range(2):
                        sc = psSC.tile([128, 2, N], F32, tag="sc", name="sc")
                        for cj in range(2):
                            c = 2 * ci + cj
                            nc.tensor.matmul(
                                sc[:, cj, :],
                                qk[p0:p0 + D, kj, c * 128:(c + 1) * 128],
                                qk[p0:p0 + D, hp, :])
                        ex = exp_.tile([128, 2, N], BF16, tag="ex", name="ex")
                        nc.scalar.activation(out=ex[:], in_=sc[:], func=AF.Exp,
                                             scale=SCALE)
                        for cj in range(2):
                            nc.tensor.matmul(av[:], v[:, 2 * ci + cj, h, :],
                                             ex[:, cj, :],
                                             start=(ci == 0 and cj == 0),
                                             stop=(ci == 1 and cj == 1))
                    nc.vector.tensor_copy(out=zr[r:r + 1, :], in_=av[D:D + 1, :])
                    nc.vector.tensor_copy(out=orw[p0:p0 + D, :], in_=av[0:D, :])
                orws.append(orw)
            # 1/Z for 4 heads via Ln+Exp (same ACT table set as softmax Exp)
            nc.scalar.activation(out=zr, in_=zr, func=AF.Ln)
            rz = zp.tile([97, N], BF16, tag="rz", bufs=1, name="rz")
            nc.scalar.activation(out=rz, in_=zr, func=AF.Exp, scale=-1.0)
            for pi in range(2):
                hp = 2 * q4 + pi
                for sub in range(2):
                    p0 = 64 * sub
                    r = 32 * (2 * pi + sub)
                    bcp = psAV.tile([64, N], F32, tag="av", name="bcp")
                    nc.tensor.matmul(bcp[:], onesb[r:r + 1, :],
                                     rz[r:r + 1, :], tile_position=(r, 0))
                    nc.vector.tensor_tensor(out=oT[p0:p0 + D, hp, :],
                                            in0=orws[pi][p0:p0 + D, :],
                                            in1=bcp[:], op=ALU.mult)

        # proj + double -> x2 (SBUF resident); proj_b folded into PE accum
        x2 = x2p.tile([128, NT, C], BF16, tag="x2", name="x2")
        for tt in range(NT):
            pr = psSC.tile([128, C], F32, tag="sc", name="pr")
            for n0, nn in ((0, 512), (512, 256)):
                for k in range(CK):
                    nc.tensor.matmul(pr[:, n0:n0 + nn],
                                     oT[:, k, tt * 128:(tt + 1) * 128],
                                     wpT[:, k, n0:n0 + nn],
                                     start=(k == 0), stop=False)
                nc.tensor.matmul(pr[:, n0:n0 + nn], onescol[:, 0:128],
                                 brow[:, 0, n0:n0 + nn], start=False, stop=True)
            nc.vector.tensor_scalar(out=x2[:, tt, :], in0=pr[:], scalar1=2.0,
                                    scalar2=None, op0=ALU.mult)
        return x2

    def mlp_front(b):
        """LN2 + h2T for item b (emitted early, during attn(b+1))."""
        x2 = x2s[b]
        h2T = hTp.tile([128, CK, N], BF16, tag="hT", name="h2T")
        for tt in range(NT):
            h2 = ln_center(x2[:, tt, :], hln, "h0", 2)
            transpose_to(h2, h2T, tt, tt + 1)
        h2Ts[b] = h2T

    def mlp_rest(b):
        t0 = b * N
        x2, h2T = x2s[b], h2Ts[b]
        for half in range(2):
            tc0 = half * 256
            g = gp.tile([128, JH, 256], BF16, tag="g", name="g")
            for j in range(JH):
                fp = psQP.tile([128, 256], F32, tag="qp", name="fp")
                for k in range(CK):
                    nc.tensor.matmul(fp[:],
                                     wf1T[:, k, j * 128:(j + 1) * 128],
                                     h2T[:, k, tc0:tc0 + 256],
                                     start=(k == 0), stop=(k == CK - 1))
                nc.scalar.activation(out=g[:, j, :], in_=fp[:], func=AF.Gelu,
                                     bias=fc1b_t[:, j:j + 1])
            for tt2 in range(2):
                tt = half * 2 + tt2
                f2 = psSC.tile([128, C], F32, tag="sc", name="f2")
                for n0, nn in ((0, 512), (512, 256)):
                    for j in range(JH):
                        nc.tensor.matmul(f2[:, n0:n0 + nn],
                                         g[:, j, tt2 * 128:(tt2 + 1) * 128],
                                         wf2T[:, j, n0:n0 + nn],
                                         start=(j == 0), stop=False)
                    nc.tensor.matmul(f2[:, n0:n0 + nn], onescol[:, 0:128],
                                     brow[:, 1, n0:n0 + nn],
                                     start=False, stop=True)
                o_t = outp.tile([128, C], F32, tag="out", name="o_t")
                nc.vector.tensor_tensor(out=o_t, in0=f2[:], in1=x2[:, tt, :],
                                        op=ALU.add)
                nc.sync.dma_start(
                    out=io["out"][t0 + tt * 128:t0 + (tt + 1) * 128, :],
                    in_=o_t)

    # ================= emission =================
    x2s = {}
    h2Ts = {}
    with tc.tile_pool(name="wstage", bufs=2) as wstage:
        # qkv + proj weights first (attn(0) needs them)
        load_wT(io["qkv_w"], 3 * C, C, wqkvT)
        load_wT(io["proj_w"], C, C, wpT)

        # qk bias = W_qk @ ln1_b  (per-feature, feature-major -> ACT bias col)
        qkb_ps = psQP.tile([128, FQK], F32, tag="qp", name="qkb_ps")
        for j in range(FQK):
            for k in range(CK):
                nc.tensor.matmul(qkb_ps[:, j:j + 1],
                                 wqkvT[:, k, j * 128:(j + 1) * 128],
                                 lnb_c[:, 0, k:k + 1],
                                 start=(k == 0), stop=(k == CK - 1))
        nc.vector.tensor_copy(out=qkb, in_=qkb_ps)
        # v bias = W_v @ ln1_b, broadcast across partitions via PE
        vb_ps = psSC.tile([1, C], F32, tag="sc", name="vb_ps")
        for k in range(CK):
            for n0, nn in ((0, 512), (512, 256)):
                nc.tensor.matmul(vb_ps[:, n0:n0 + nn], lnb_c[:, 0, k:k + 1],
                                 wqkvT[:, k, 2 * C + n0:2 * C + n0 + nn],
                                 start=(k == 0), stop=(k == CK - 1))
        nc.vector.tensor_copy(out=vrow, in_=vb_ps)
        fold_lnw(wqkvT, 3 * C, ln1w_c)

        x2s[0] = attn(0)

        # fc weights (overlap with attn(0) compute)
        load_wT(io["fc1_w"], HID, C, wf1T)
        load_wT(io["fc2_w"], C, HID, wf2T)
        # fc1 bias' = fc1_b + W1 @ ln2_b  (hidden-major col)
        f1b_ps = psQP.tile([128, JH], F32, tag="qp", name="f1b_ps")
        for j in range(JH):
            for k in range(CK):
                nc.tensor.matmul(f1b_ps[:, j:j + 1],
                                 wf1T[:, k, j * 128:(j + 1) * 128],
                                 lnb_c[:, 1, k:k + 1],
                                 start=(k == 0), stop=(k == CK - 1))
        f1b_st = consts.tile([128, JH], F32, tag="f1b_st", name="f1b_st")
        nc.sync.dma_start(out=f1b_st,
                          in_=io["fc1_b"].rearrange("(j p) -> p j", p=128))
        nc.vector.tensor_tensor(out=fc1b_t, in0=f1b_ps, in1=f1b_st, op=ALU.add)
        fold_lnw(wf1T, HID, ln2w_c)

        x2s[1] = attn(1, h2T_of=0)
        mlp_rest(0)
        x2s[2] = attn(2, h2T_of=1)
        mlp_rest(1)
        x2s[3] = attn(3, h2T_of=2)
        mlp_rest(2)
        mlp_front(3)
        mlp_rest(3)


_CACHE = {}


def _build():
    if "nc" in _CACHE:
        return _CACHE["nc"]
    nc = bacc.Bacc("TRN2", target_bir_lowering=False, debug=False,
                   num_devices=NCORES)
    io = {}
    io["x"] = nc.dram_tensor("x", [T, C], F32, kind="ExternalInput").ap()
    for name, shape in [("ln1_w", [C]), ("ln1_b", [C]), ("qkv_w", [3 * C, C]),
                        ("proj_w", [C, C]), ("proj_b", [C]), ("ln2_w", [C]),
                        ("ln2_b", [C]), ("fc1_w", [HID, C]), ("fc1_b", [HID]),
                        ("fc2_w", [C, HID]), ("fc2_b", [C])]:
        io[name] = nc.dram_tensor(name, shape, F32, kind="ExternalInput").ap()
    io["out"] = nc.dram_tensor("out", [T, C], F32, kind="ExternalOutput").ap()

    with tile.TileContext(nc) as tc:
        with ExitStack() as ctx:
            _emit(tc, io, ctx)
    nc.compile()
    _CACHE["nc"] = nc
    return nc


def kernel(**inputs):
    nc = _build()
    arrs = {k: np.ascontiguousarray(np.asarray(v, dtype=np.float32))
            for k, v in inputs.items()}
    x = arrs.pop("x").reshape(B, N, C)
    in_maps = []
    for c in range(NCORES):
        m = dict(arrs)
        m["x"] = np.ascontiguousarray(x[c * BPC:(c + 1) * BPC].reshape(T, C))
        in_maps.append(m)
    res = run_bass_kernel_spmd(nc, in_maps, core_ids=list(range(NCORES)))
    out = np.concatenate(
        [r["out"].reshape(BPC, N, C) for r in res.results], axis=0)
    return out.astype(np.float32)


if __name__ == "__main__":
    rng = np.random.default_rng(0)
    ins = {
        "x": rng.standard_normal((B, N, C), dtype=np.float32),
        "ln1_w": np.ones(C, np.float32), "ln1_b": np.zeros(C, np.float32),
        "qkv_w": rng.standard_normal((3 * C, C), dtype=np.float32) / np.sqrt(C),
        "proj_w": rng.standard_normal((C, C), dtype=np.float32) / np.sqrt(C),
        "proj_b": np.zeros(C, np.float32),
        "ln2_w": np.ones(C, np.float32), "ln2_b": np.zeros(C, np.float32),
        "fc1_w": rng.standard_normal((HID, C), dtype=np.float32) / np.sqrt(C),
        "fc1_b": np.zeros(HID, np.float32),
        "fc2_w": rng.standard_normal((C, HID), dtype=np.float32) / np.sqrt(HID),
        "fc2_b": np.zeros(C, np.float32),
    }
    out = kernel(**ins)
    print("out", out.shape, out.dtype, np.abs(out).max())


# revision 23
# speedup vs baseline: 1.2248x; 1.0154x over previous
"""TRN2 Bass kernel: transformer Block (LN->MHA->2x residual->LN->MLP) for
B=32,N=512,C=768,H=12. Data-parallel over batch across 8 NeuronCores (4
items/core). Fully fused per-item pipeline, all weights resident in SBUF as
bf16, no DRAM spills.

Per-core program:
  prologue: PE-transpose all weights into [c-on-partition] bf16 layout; fold
    ln1_w/ln2_w into qkv_w/fc1_w; fold ln biases into matmul biases.
  per item b (software-pipelined: attn(b+1) emitted before mlp_rest(b)):
    LN1 -> h0 (bf16) -> PE-transpose -> qk/v matmuls -> per-head
    scoresT = kT.T@qT -> exp (ScalarE, 1024-wide from PSUM, no max-sub) ->
    [v|1]-augmented AV matmul -> denom reciprocal via Ln+Exp on ScalarE
    (same ACT table set as Exp) -> PE-broadcast -> oT -> proj ->
    x2=2*(proj+proj_b) in SBUF -> LN2 -> h2T -> per-256-token-half:
    fc1 -> gelu -> fc2 -> + x2 + fc2_b -> out
"""
import numpy as np
from contextlib import ExitStack

import concourse.bass as bass
import concourse.tile as tile
import concourse.bacc as bacc
from concourse import mybir
from concourse.bass_utils import run_bass_kernel_spmd
from concourse.masks import make_identity

F32 = mybir.dt.float32
BF16 = mybir.dt.bfloat16
AF = mybir.ActivationFunctionType
ALU = mybir.AluOpType

B, N, C = 32, 512, 768
H, D = 12, 64
HID = 4 * C
EPS = 1e-5
NCORES = 8
BPC = B // NCORES            # batch items per core
T = BPC * N                  # tokens per core
CK = C // 128                # 6 contraction chunks over C
FQK = (2 * C) // 128         # 12 feature tiles for q+k
JH = HID // 128              # 24 hidden feature tiles
NT = N // 128                # 4 token tiles per item
SCALE = D ** -0.5


def _bc(ap, p=128):
    """Broadcast a 1-D DRAM AP across p partitions (stride-0 partition dim)."""
    return bass.AP(tensor=ap.tensor, offset=ap.offset, ap=[[0, p]] + list(ap.ap))


def _emit(tc, io, ctx):
    nc = tc.nc

    consts = ctx.enter_context(tc.tile_pool(name="consts", bufs=1))
    wbig = ctx.enter_context(tc.tile_pool(name="wbig", bufs=1))
    small = ctx.enter_context(tc.tile_pool(name="small", bufs=4))
    xio = ctx.enter_context(tc.tile_pool(name="xio", bufs=2))
    hln = ctx.enter_context(tc.tile_pool(name="hln", bufs=2))
    hTp = ctx.enter_context(tc.tile_pool(name="hTp", bufs=3))
    qkp = ctx.enter_context(tc.tile_pool(name="qkp", bufs=1))
    vp_ = ctx.enter_context(tc.tile_pool(name="vp", bufs=1))
    exp_ = ctx.enter_context(tc.tile_pool(name="exp", bufs=2))
    zp = ctx.enter_context(tc.tile_pool(name="zp", bufs=2))
    orp = ctx.enter_context(tc.tile_pool(name="orp", bufs=2))
    x2p = ctx.enter_context(tc.tile_pool(name="x2p", bufs=2))
    gp = ctx.enter_context(tc.tile_pool(name="gp", bufs=1))
    outp = ctx.enter_context(tc.tile_pool(name="outp", bufs=2))
    psQP = ctx.enter_context(tc.tile_pool(name="psQP", bufs=2, space="PSUM"))
    psSC = ctx.enter_context(tc.tile_pool(name="psSC", bufs=2, space="PSUM"))
    psAV = ctx.enter_context(tc.tile_pool(name="psAV", bufs=2, space="PSUM"))

    # ---------------- constants ----------------
    ident32 = consts.tile([128, 128], F32)
    make_identity(nc, ident32)
    identb = consts.tile([128, 128], BF16)
    nc.vector.tensor_copy(out=identb, in_=ident32)
    onesb = consts.tile([97, 64], BF16)
    nc.vector.memset(onesb, 1.0)
    onescol = consts.tile([1, 128], BF16)
    nc.vector.memset(onescol, 1.0)
    onecol = consts.tile([128, NT * H], BF16)
    nc.vector.memset(onecol, 1.0)
    epst = consts.tile([128, 1], F32)
    nc.vector.memset(epst, EPS)

    # bias rows [1, C] (folded into PE accumulations via rank-1 ones-matmuls)
    brow = consts.tile([1, 2, C], BF16)
    for bi, bname in ((0, "proj_b"), (1, "fc2_b")):
        b_st = xio.tile([128, C], F32, tag="xio", name="b_st")
        nc.sync.dma_start(out=b_st[0:1, :], in_=_bc(io[bname], p=1))
        nc.vector.tensor_copy(out=brow[:, bi, :], in_=b_st[0:1, :])

    fc1b_t = consts.tile([128, JH], F32)
    ln1w_c = consts.tile([128, CK], F32)
    nc.sync.dma_start(out=ln1w_c, in_=io["ln1_w"].rearrange("(k p) -> p k", p=128))
    ln2w_c = consts.tile([128, CK], F32)
    nc.sync.dma_start(out=ln2w_c, in_=io["ln2_w"].rearrange("(k p) -> p k", p=128))
    lnb_st = consts.tile([128, 2, CK], F32)
    nc.sync.dma_start(out=lnb_st[:, 0, :],
                      in_=io["ln1_b"].rearrange("(k p) -> p k", p=128))
    nc.sync.dma_start(out=lnb_st[:, 1, :],
                      in_=io["ln2_b"].rearrange("(k p) -> p k", p=128))
    lnb_c = consts.tile([128, 2, CK], BF16)
    nc.vector.tensor_copy(out=lnb_c, in_=lnb_st)
    qkb = consts.tile([128, FQK], F32)
    vrow = consts.tile([1, C], BF16)

    # resident weights (bf16, contraction dim on partitions)
    wqkvT = wbig.tile([128, CK, 3 * C], BF16)
    wpT = wbig.tile([128, CK, C], BF16)
    wf1T = wbig.tile([128, CK, HID], BF16)
    wf2T = wbig.tile([128, JH, C], BF16)

    evac_ctr = [0]

    def load_wT(w_ap, nrows, ncols, dst):
        """w [nrows, ncols] row-major DRAM -> dst [128, ncols//128, nrows] BF16."""
        nj, nk = nrows // 128, ncols // 128
        wr = w_ap.rearrange("(j p) c -> p j c", p=128)
        for j in range(nj):
            for c0 in range(0, nk, 6):
                cn = min(6, nk - c0)
                piece = wstage.tile([128, 768], F32, tag="wstage", name="piece")
                nc.sync.dma_start(out=piece[:, 0:cn * 128],
                                  in_=wr[:, j, c0 * 128:(c0 + cn) * 128])
                for b0 in range(0, cn, 4):
                    bn_ = min(4, cn - b0)
                    tp = psQP.tile([128, bn_, 128], F32, tag="qp", name="tp")
                    for k in range(bn_):
                        nc.tensor.matmul(
                            tp[:, k, :], piece[:, (b0 + k) * 128:(b0 + k + 1) * 128],
                            ident32[:], is_transpose=True)
                    dstap = dst[:, c0 + b0:c0 + b0 + bn_, j * 128:(j + 1) * 128]
                    if evac_ctr[0] % 2 == 0:
                        nc.vector.tensor_copy(out=dstap, in_=tp[:])
                    else:
                        nc.scalar.copy(out=dstap, in_=tp[:])
                    evac_ctr[0] += 1

    def fold_lnw(dst, nf, lnw_col):
        """dst[:, k, :] *= lnw_col[:, k] for all k (per-partition scalar)."""
        for k in range(CK):
            nc.vector.tensor_scalar(out=dst[:, k, 0:nf], in0=dst[:, k, 0:nf],
                                    scalar1=lnw_col[:, k:k + 1], scalar2=None,
                                    op0=ALU.mult)

    # ---------------- per-item helpers ----------------
    def ln_stats(x_t, mv_out):
        """Accumulate mean/var of x_t [128, C] into mv_out [128, 2]."""
        st = small.tile([128, 3, nc.vector.BN_STATS_DIM], F32, tag="bnst",
                        name="st")
        for i in range(3):
            nc.vector.bn_stats(out=st[:, i, :], in_=x_t[:, 256 * i:256 * (i + 1)])
        nc.vector.bn_aggr(out=mv_out, in_=st)

    def ln_rstd(mv, rstd_out, n):
        """rstd_out [128, n] = (var + eps)^-0.5 via Ln+Exp (exp table set)."""
        lnv = small.tile([128, n], F32, tag="lnv", name="lnv")
        nc.scalar.activation(out=lnv, in_=mv[:, 0:n, 1], func=AF.Ln, bias=epst)
        nc.scalar.activation(out=rstd_out, in_=lnv, func=AF.Exp, scale=-0.5)

    def ln_scale(x_t, mu, rstd, pool, tag, bufs):
        h = pool.tile([128, C], BF16, tag=tag, bufs=bufs, name="h")
        nc.vector.tensor_scalar(out=h, in0=x_t, scalar1=mu,
                                scalar2=rstd, op0=ALU.subtract, op1=ALU.mult)
        return h

    def transpose_to(h, dstT, tt, who):
        """h [128, C] BF16 -> dstT[:, k, tt*128:(tt+1)*128] for k in CK."""
        for b0, bn_ in ((0, 4), (4, 2)):
            tp = psQP.tile([128, bn_, 128], BF16, tag="qp", name="tpb")
            for k in range(bn_):
                nc.tensor.matmul(tp[:, k, :],
                                 h[:, (b0 + k) * 128:(b0 + k + 1) * 128],
                                 identb[:], is_transpose=True)
            dstap = dstT[:, b0:b0 + bn_, tt * 128:(tt + 1) * 128]
            if who % 2 == 0:
                nc.vector.tensor_copy(out=dstap, in_=tp[:])
            else:
                nc.scalar.copy(out=dstap, in_=tp[:])

    def attn(b, h2T_of=None):
        """Attention for item b; also emits LN2+h2T for item h2T_of."""
        t0 = b * N
        h0T = hTp.tile([128, CK, N], BF16, tag="hT", name="h0T")
        for tt in range(NT):
            x_t = xio.tile([128, C], F32, tag="xio", name="x_t")
            nc.sync.dma_start(
                out=x_t, in_=io["x"][t0 + tt * 128:t0 + (tt + 1) * 128, :])
            idx = b * NT + tt
            h = ln_scale(x_t, mv16[:, idx, 0:1], rstd16[:, idx:idx + 1],
                         hln, "h0", 2)
            transpose_to(h, h0T, tt, tt)

        # qT/kT feature-major: tile j holds heads 2j/2j+1 stacked on partitions
        qk = qkp.tile([128, FQK, N], BF16, tag="qk", name="qk")
        for j in range(FQK):
            qp = psQP.tile([128, N], F32, tag="qp", name="qp")
            for k in range(CK):
                nc.tensor.matmul(qp[:], wqkvT[:, k, j * 128:(j + 1) * 128],
                                 h0T[:, k, :], start=(k == 0),
                                 stop=(k == CK - 1))
            if j % 2 == 0:
                nc.scalar.activation(out=qk[:, j, :], in_=qp[:],
                                     func=AF.Identity, bias=qkb[:, j:j + 1])
            else:
                nc.vector.tensor_scalar(out=qk[:, j, :], in0=qp[:],
                                        scalar1=qkb[:, j:j + 1], scalar2=None,
                                        op0=ALU.add)

        # v token-major with ones column at d=D
        v = vp_.tile([128, NT, H, D + 1], BF16, tag="v", name="v")
        nc.vector.tensor_copy(
            out=v[:, :, :, D:D + 1],
            in_=onecol.rearrange("p (a b c) -> p a b c", a=NT, b=H))
        for tt in range(NT):
            for n0, nn in ((0, 512), (512, 256)):
                vps = psQP.tile([128, nn], F32, tag="qp", name="vps")
                for k in range(CK):
                    nc.tensor.matmul(vps[:],
                                     h0T[:, k, tt * 128:(tt + 1) * 128],
                                     wqkvT[:, k, 2 * C + n0:2 * C + n0 + nn],
                                     start=(k == 0), stop=False)
                nc.tensor.matmul(vps[:], onescol[:, 0:128],
                                 vrow[:, n0:n0 + nn], start=False, stop=True)
                nc.vector.tensor_copy(
                    out=v[:, tt, n0 // D:(n0 + nn) // D, 0:D],
                    in_=vps.rearrange("p (h d) -> p h d", d=D))

        if h2T_of is not None:
            mlp_front(h2T_of)

        # attention: head h -> oT chunk h//2, partitions 64*(h%2)
        oT = hTp.tile([128, CK, N], BF16, tag="hT", name="oT")
        for q4 in range(H // 4):
            zr = zp.tile([97, N], F32, tag="zr", name="zr")
            orws = []
            for pi in range(2):
                hp = 2 * q4 + pi
                kj = FQK // 2 + hp
                orw = orp.tile([128, N], BF16, tag="orw", name="orw")
                for sub in range(2):
                    h = 2 * hp + sub
                    p0 = 64 * sub
                    r = 32 * (2 * pi + sub)
                    av = psAV.tile([D + 1, N], F32, tag="av", name="av")
                    for ci in range(2):
                        sc = psSC.tile([128, 2, N], F32, tag="sc", name="sc")
                        for cj in range(2):
                            c = 2 * ci + cj
                            nc.tensor.matmul(
                                sc[:, cj, :],
                                qk[p0:p0 + D, kj, c * 128:(c + 1) * 128],
                                qk[p0:p0 + D, hp, :])
                        ex = exp_.tile([128, 2, N], BF16, tag="ex", name="ex")
                        nc.scalar.activation(out=ex[:], in_=sc[:], func=AF.Exp,
                                             scale=SCALE)
                        for cj in range(2):
                            nc.tensor.matmul(av[:], v[:, 2 * ci + cj, h, :],
                                             ex[:, cj, :],
                                             start=(ci == 0 and cj == 0),
                                             stop=(ci == 1 and cj == 1))
                    nc.vector.tensor_copy(out=zr[r:r + 1, :], in_=av[D:D + 1, :])
                    nc.vector.tensor_copy(out=orw[p0:p0 + D, :], in_=av[0:D, :])
                orws.append(orw)
            # 1/Z for 4 heads via Ln+Exp (same ACT table set as softmax Exp)
            nc.scalar.activation(out=zr, in_=zr, func=AF.Ln)
            rz = zp.tile([97, N], BF16, tag="rz", bufs=1, name="rz")
            nc.scalar.activation(out=rz, in_=zr, func=AF.Exp, scale=-1.0)
            for pi in range(2):
                hp = 2 * q4 + pi
                for sub in range(2):
                    p0 = 64 * sub
                    r = 32 * (2 * pi + sub)
                    bcp = psAV.tile([64, N], F32, tag="av", name="bcp")
                    nc.tensor.matmul(bcp[:], onesb[r:r + 1, :],
                                     rz[r:r + 1, :], tile_position=(r, 0))
                    nc.vector.tensor_tensor(out=oT[p0:p0 + D, hp, :],
                                            in0=orws[pi][p0:p0 + D, :],
                                            in1=bcp[:], op=ALU.mult)

        # proj + double -> x2 (SBUF resident); proj_b folded into PE accum
        x2 = x2p.tile([128, NT, C], BF16, tag="x2", name="x2")
        for tt in range(NT):
            pr = psSC.tile([128, C], F32, tag="sc", name="pr")
            for n0, nn in ((0, 512), (512, 256)):
                for k in range(CK):
                    nc.tensor.matmul(pr[:, n0:n0 + nn],
                                     oT[:, k, tt * 128:(tt + 1) * 128],
                                     wpT[:, k, n0:n0 + nn],
                                     start=(k == 0), stop=False)
                nc.tensor.matmul(pr[:, n0:n0 + nn], onescol[:, 0:128],
                                 brow[:, 0, n0:n0 + nn], start=False, stop=True)
            nc.vector.tensor_scalar(out=x2[:, tt, :], in0=pr[:], scalar1=2.0,
                                    scalar2=None, op0=ALU.mult)
        return x2

    def mlp_front(b):
        """LN2 + h2T for item b (emitted in the exp phase: Ln/Exp in-set)."""
        x2 = x2s[b]
        mv2 = small.tile([128, NT, 2], F32, tag="mv2", name="mv2")
        for tt in range(NT):
            ln_stats(x2[:, tt, :], mv2[:, tt, :])
        rstd2 = small.tile([128, NT], F32, tag="rstd2", name="rstd2")
        ln_rstd(mv2, rstd2, NT)
        h2T = hTp.tile([128, CK, N], BF16, tag="hT", name="h2T")
        for tt in range(NT):
            h2 = ln_scale(x2[:, tt, :], mv2[:, tt, 0:1], rstd2[:, tt:tt + 1],
                          hln, "h0", 2)
            transpose_to(h2, h2T, tt, tt + 1)
        h2Ts[b] = h2T

    def mlp_rest(b):
        t0 = b * N
        x2, h2T = x2s[b], h2Ts[b]
        for half in range(2):
            tc0 = half * 256
            g = gp.tile([128, JH, 256], BF16, tag="g", name="g")
            for j in range(JH):
                fp = psQP.tile([128, 256], F32, tag="qp", name="fp")
                for k in range(CK):
                    nc.tensor.matmul(fp[:],
                                     wf1T[:, k, j * 128:(j + 1) * 128],
                                     h2T[:, k, tc0:tc0 + 256],
                                     start=(k == 0), stop=(k == CK - 1))
                nc.scalar.activation(out=g[:, j, :], in_=fp[:], func=AF.Gelu,
                                     bias=fc1b_t[:, j:j + 1])
            for tt2 in range(2):
                tt = half * 2 + tt2
                f2 = psSC.tile([128, C], F32, tag="sc", name="f2")
                for n0, nn in ((0, 512), (512, 256)):
                    for j in range(JH):
                        nc.tensor.matmul(f2[:, n0:n0 + nn],
                                         g[:, j, tt2 * 128:(tt2 + 1) * 128],
                                         wf2T[:, j, n0:n0 + nn],
                                         start=(j == 0), stop=False)
                    nc.tensor.matmul(f2[:, n0:n0 + nn], onescol[:, 0:128],
                                     brow[:, 1, n0:n0 + nn],
                                     start=False, stop=True)
                o_t = outp.tile([128, C], F32, tag="out", name="o_t")
                nc.vector.tensor_tensor(out=o_t, in0=f2[:], in1=x2[:, tt, :],
                                        op=ALU.add)
                nc.sync.dma_start(
                    out=io["out"][t0 + tt * 128:t0 + (tt + 1) * 128, :],
                    in_=o_t)

    # ================= emission =================
    x2s = {}
    h2Ts = {}
    mv16 = consts.tile([128, BPC * NT, 2], F32, tag="mv16", name="mv16")
    rstd16 = consts.tile([128, BPC * NT], F32, tag="rstd16", name="rstd16")
    with tc.tile_pool(name="wstage", bufs=2) as wstage:
        # LN1 stats front-pass: all rstd values via ONE batched Ln+Exp so the
        # attention phase needs no table-sensitive ACT ops besides Exp.
        for idx in range(BPC * NT):
            x_t = xio.tile([128, C], F32, tag="xio", name="x_t")
            nc.sync.dma_start(
                out=x_t, in_=io["x"][idx * 128:(idx + 1) * 128, :])
            ln_stats(x_t, mv16[:, idx, :])
        ln_rstd(mv16, rstd16, BPC * NT)
        # qkv + proj weights first (attn(0) needs them)
        load_wT(io["qkv_w"], 3 * C, C, wqkvT)
        load_wT(io["proj_w"], C, C, wpT)

        # qk bias = W_qk @ ln1_b  (per-feature, feature-major -> ACT bias col)
        qkb_ps = psQP.tile([128, FQK], F32, tag="qp", name="qkb_ps")
        for j in range(FQK):
            for k in range(CK):
                nc.tensor.matmul(qkb_ps[:, j:j + 1],
                                 wqkvT[:, k, j * 128:(j + 1) * 128],
                                 lnb_c[:, 0, k:k + 1],
                                 start=(k == 0), stop=(k == CK - 1))
        nc.vector.tensor_copy(out=qkb, in_=qkb_ps)
        # v bias = W_v @ ln1_b, broadcast across partitions via PE
        vb_ps = psSC.tile([1, C], F32, tag="sc", name="vb_ps")
        for k in range(CK):
            for n0, nn in ((0, 512), (512, 256)):
                nc.tensor.matmul(vb_ps[:, n0:n0 + nn], lnb_c[:, 0, k:k + 1],
                                 wqkvT[:, k, 2 * C + n0:2 * C + n0 + nn],
                                 start=(k == 0), stop=(k == CK - 1))
        nc.vector.tensor_copy(out=vrow, in_=vb_ps)
        fold_lnw(wqkvT, 3 * C, ln1w_c)

        x2s[0] = attn(0)

        # fc weights (overlap with attn(0) compute)
        load_wT(io["fc1_w"], HID, C, wf1T)
        load_wT(io["fc2_w"], C, HID, wf2T)
        # fc1 bias' = fc1_b + W1 @ ln2_b  (hidden-major col)
        f1b_ps = psQP.tile([128, JH], F32, tag="qp", name="f1b_ps")
        for j in range(JH):
            for k in range(CK):
                nc.tensor.matmul(f1b_ps[:, j:j + 1],
                                 wf1T[:, k, j * 128:(j + 1) * 128],
                                 lnb_c[:, 1, k:k + 1],
                                 start=(k == 0), stop=(k == CK - 1))
        f1b_st = consts.tile([128, JH], F32, tag="f1b_st", name="f1b_st")
        nc.sync.dma_start(out=f1b_st,
                          in_=io["fc1_b"].rearrange("(j p) -> p j", p=128))
        nc.vector.tensor_tensor(out=fc1b_t, in0=f1b_ps, in1=f1b_st, op=ALU.add)
        fold_lnw(wf1T, HID, ln2w_c)

        # phase-grouped: [attn0,attn1][mlp0,mlp1][attn2,attn3][mlp2,mlp3]
        # so exp-set and gelu-set ACT ops only alternate at phase boundaries
        x2s[1] = attn(1, h2T_of=0)
        mlp_front(1)
        mlp_rest(0)
        mlp_rest(1)
        x2s[2] = attn(2)
        x2s[3] = attn(3, h2T_of=2)
        mlp_front(3)
        mlp_rest(2)
        mlp_rest(3)


_CACHE = {}


def _build():
    if "nc" in _CACHE:
        return _CACHE["nc"]
    nc = bacc.Bacc("TRN2", target_bir_lowering=False, debug=False,
                   num_devices=NCORES)
    io = {}
    io["x"] = nc.dram_tensor("x", [T, C], F32, kind="ExternalInput").ap()
    for name, shape in [("ln1_w", [C]), ("ln1_b", [C]), ("qkv_w", [3 * C, C]),
                        ("proj_w", [C, C]), ("proj_b", [C]), ("ln2_w", [C]),
                        ("ln2_b", [C]), ("fc1_w", [HID, C]), ("fc1_b", [HID]),
                        ("fc2_w", [C, HID]), ("fc2_b", [C])]:
        io[name] = nc.dram_tensor(name, shape, F32, kind="ExternalInput").ap()
    io["out"] = nc.dram_tensor("out", [T, C], F32, kind="ExternalOutput").ap()

    with tile.TileContext(nc) as tc:
        with ExitStack() as ctx:
            _emit(tc, io, ctx)
    nc.compile()
    _CACHE["nc"] = nc
    return nc


def kernel(**inputs):
    nc = _build()
    arrs = {k: np.ascontiguousarray(np.asarray(v, dtype=np.float32))
            for k, v in inputs.items()}
    x = arrs.pop("x").reshape(B, N, C)
    in_maps = []
    for c in range(NCORES):
        m = dict(arrs)
        m["x"] = np.ascontiguousarray(x[c * BPC:(c + 1) * BPC].reshape(T, C))
        in_maps.append(m)
    res = run_bass_kernel_spmd(nc, in_maps, core_ids=list(range(NCORES)))
    out = np.concatenate(
        [r["out"].reshape(BPC, N, C) for r in res.results], axis=0)
    return out.astype(np.float32)


if __name__ == "__main__":
    rng = np.random.default_rng(0)
    ins = {
        "x": rng.standard_normal((B, N, C), dtype=np.float32),
        "ln1_w": np.ones(C, np.float32), "ln1_b": np.zeros(C, np.float32),
        "qkv_w": rng.standard_normal((3 * C, C), dtype=np.float32) / np.sqrt(C),
        "proj_w": rng.standard_normal((C, C), dtype=np.float32) / np.sqrt(C),
        "proj_b": np.zeros(C, np.float32),
        "ln2_w": np.ones(C, np.float32), "ln2_b": np.zeros(C, np.float32),
        "fc1_w": rng.standard_normal((HID, C), dtype=np.float32) / np.sqrt(C),
        "fc1_b": np.zeros(HID, np.float32),
        "fc2_w": rng.standard_normal((C, HID), dtype=np.float32) / np.sqrt(HID),
        "fc2_b": np.zeros(C, np.float32),
    }
    out = kernel(**ins)
    print("out", out.shape, out.dtype, np.abs(out).max())
